# revision 29
# baseline (speedup 1.0000x reference)
"""OCS fused kernel for Trainium2, data-parallel over batch across 8 cores.

v2 restructuring (validated in f64 vs reference, see sim_v2.py):

All branches fold into per-chunk PSUM accumulation of six bf16 matmuls:
    ypre = B3''@x + A2d@t1 + A2d@t2 + Wdm@m2h' + Wdm@m2v' + C2@h1 + bout
with
    t1 = x(-1)+x(+1), t2 = x(-128)+x(+128)      (flat shifts, zero-pad)
    mle = min(x, x(-1)) [c=0 patched to min(x, x(+1))]
    m2h' = mle + mle(+1)  with c=127 / x-term edge patches
    mu  = min(x, x(-128)); m2v' = mu + mu(+128) with r=0/127 patches
using |a-b| = a+b-2min(a,b) to turn the 4-neighbor abs-diff branch into
two running-min arrays, and
    B3'' = W_proj(W_out.w1)W_in + W_proj + 4Wd,  Wd = 0.25*dwt*W_proj
    A2d  = A2 + Wd,  Wdm = -2Wd,  A2 = 0.25*W_proj(W_out.(w0+w2))W_in
plus two small A2 wrap matmuls for the column-scan orders.
Channel branch: rank-1 m = g g^T collapses to 3 [C,32] matmuls on
shifted x (weights built on-device from g = sum of x), silu, C2 matmul.
BatchNorm: per-core (sum, sumsq) -> 1KB AllReduce -> affine; output bf16.
"""

import numpy as np
import ml_dtypes

B, C, Himg, Wimg = 8, 128, 128, 128
L = Himg * Wimg            # 16384
NCORES = 8
NCH = 512                  # psum chunk columns (one bank)
NCHUNK = L // NCH          # 32
NPAIR = NCHUNK // 2        # 16 evacuation pairs
NW = 2048                  # window columns (4 chunks)
NGRP = L // NW             # 8
EPS_BN = 1e-5
NTOT = float(B * L)

_CACHE = {}


def _make_patched_tc():
    """TileContext whose exit drain splits sem waits one-per-Drain.

    The walrus build in this container rejects Drain instructions carrying
    more than one sem wait ("Too many sync wait commands"). Stock
    TileContext attaches the whole global vector clock to a single tail
    Drain; emit one Drain per outstanding proc instead.
    """
    import bass_rust
    import concourse.tile as tile
    from concourse.vector_clock import ScopedClock

    class PatchedTC(tile.TileContext):
        def _drain_and_barrier(self, tick_clock, wait_clock):
            gc = list(tick_clock.global_clock)
            for i, v in enumerate(gc):
                if v:
                    single = [0] * len(gc)
                    single[i] = v
                    d = self.nc.sync.drain()
                    wait_clock.add_sem_waits(
                        d.ins, ScopedClock({None: bass_rust.VectorClock(single)})
                    )
            self.nc.all_engine_barrier()
            assert self.sems is not None
            popped = self.nc._tile_sem_poison_stack.pop()
            assert popped is self._sem_poison
            self.nc.clear_and_free_semaphores(list(self.sems.allocated().values()))
            self.nc.all_engine_barrier()

    return PatchedTC


def _split_excess_waits(nc):
    """Walrus here allows one sem wait per instruction; hoist extras onto
    same-engine NoOps inserted immediately before the instruction."""
    import bass_rust

    nid = 0
    for blk in nc.main_func.blocks:
        out = []
        for ins in blk.instructions:
            si = getattr(ins, "sync_info", None)
            waits = list(si.on_wait) if si is not None else []
            if len(waits) > 1:
                for w in waits[:-1]:
                    nid += 1
                    nop = bass_rust.InstNoOp(
                        name=f"I-waitsplit-{nid}", ins=[], outs=[])
                    nop.engine = ins.engine
                    nop.sync_info = bass_rust.SyncInfo(
                        on_wait=[w], on_update=[])
                    nc.register_instruction(nop, overwrite=True)
                    out.append(nop)
                si.on_wait = [waits[-1]]
                ins.sync_info = si
            out.append(ins)
        blk.instructions = out


def _build_program():
    import concourse.bass as bass
    import concourse.mybir as mybir

    PatchedTC = _make_patched_tc()

    f32 = mybir.dt.float32
    bf16 = mybir.dt.bfloat16
    Alu = mybir.AluOpType
    Act = mybir.ActivationFunctionType

    nc = bass.Bass(target_bir_lowering=False, num_devices=NCORES)

    x_ext = nc.declare_dram_parameter("x", [C, L], bf16, isOutput=False)
    wb3t_ext = nc.declare_dram_parameter("wb3t", [C, C], bf16, isOutput=False)
    wa2dt_ext = nc.declare_dram_parameter("wa2dt", [C, C], bf16, isOutput=False)
    wa2t_ext = nc.declare_dram_parameter("wa2t", [C, C], bf16, isOutput=False)
    wdmt_ext = nc.declare_dram_parameter("wdmt", [C, C], bf16, isOutput=False)
    c2t4_ext = nc.declare_dram_parameter("c2t4", [C, C], bf16, isOutput=False)
    wcho_ext = nc.declare_dram_parameter("wcho", [C, C], f32, isOutput=False)
    wchi_ext = nc.declare_dram_parameter("wchi", [C, C], f32, isOutput=False)
    wm1t_ext = nc.declare_dram_parameter("wm1t", [C, 32], f32, isOutput=False)
    taps_ext = nc.declare_dram_parameter("taps", [C, 3], f32, isOutput=False)
    b1t_ext = nc.declare_dram_parameter("b1t", [C, 1], f32, isOutput=False)
    bout_ext = nc.declare_dram_parameter("bout", [C, 1], f32, isOutput=False)
    gb_ext = nc.declare_dram_parameter("gb", [C, 2], f32, isOutput=False)
    y_ext = nc.declare_dram_parameter("y", [C, L], bf16, isOutput=True)

    with PatchedTC(nc) as tc:
        with (
            tc.tile_pool(name="wp", bufs=1) as wp,
            tc.tile_pool(name="big", bufs=1) as big,
            tc.tile_pool(name="win", bufs=4) as win,
            tc.tile_pool(name="wmid", bufs=2) as wmid,
            tc.tile_pool(name="sm", bufs=1) as sm,
            tc.tile_pool(name="dump", bufs=2) as dump,
            tc.tile_pool(name="yps", bufs=3, space="PSUM") as yps,
            tc.tile_pool(name="hps", bufs=1, space="PSUM") as hps,
            tc.tile_pool(name="sps", bufs=1, space="PSUM") as sps,
            tc.tile_pool(name="dram", bufs=1, space="DRAM") as dram,
        ):
            # ---- weights to SBUF ----
            wb3t = wp.tile([C, C], bf16)
            wa2dt = wp.tile([C, C], bf16)
            wa2t = wp.tile([C, C], bf16)
            wdmt = wp.tile([C, C], bf16)
            c2t4 = wp.tile([C, C], bf16)
            wcho = wp.tile([C, C], f32)
            wchi = wp.tile([C, C], f32)
            wm1t = wp.tile([C, 32], f32)
            taps = wp.tile([C, 3], f32)
            b1t = wp.tile([C, 1], f32)
            bout = wp.tile([C, 1], f32)
            gb = wp.tile([C, 2], f32)
            for t, e in [(wb3t, wb3t_ext), (wa2dt, wa2dt_ext),
                         (wa2t, wa2t_ext), (wdmt, wdmt_ext),
                         (c2t4, c2t4_ext), (wcho, wcho_ext), (wchi, wchi_ext),
                         (wm1t, wm1t_ext), (taps, taps_ext), (b1t, b1t_ext),
                         (bout, bout_ext), (gb, gb_ext)]:
                nc.gpsimd.dma_start(out=t, in_=e[:])

            # ---- big SBUF arrays ----
            xbf = big.tile([C, L], bf16)
            ypre = big.tile([C, L], bf16)
            h1sb = big.tile([C, NGRP * NCH], bf16)

            gsums = sm.tile([C, NGRP // 2], f32)
            ysum = sm.tile([C, NPAIR], f32)
            ysq = sm.tile([C, NPAIR], f32)

            # ---- dummy collective: warm CC rings, absorb launch skew ----
            warm = sm.tile([C, 2], f32)
            nc.vector.memset(warm, 0.0)
            wcc_in = dram.tile([C, 2], f32)
            wcc_out = dram.tile([C, 2], f32)
            nc.gpsimd.dma_start(out=wcc_in[:], in_=warm)
            nc.gpsimd.collective_compute(
                "AllReduce", Alu.add,
                replica_groups=[list(range(NCORES))],
                ins=[wcc_in.opt()], outs=[wcc_out.opt()])

            # ---- load x windows; row-sum accum on ACT (2 windows/op) ----
            for g in range(NGRP):
                lo, hi = g * NW, (g + 1) * NW
                nc.sync.dma_start(out=xbf[:, lo:hi], in_=x_ext[:, lo:hi])
                if g % 2 == 1:
                    gdump = dump.tile([C, 2 * NW], bf16, tag="gs")
                    nc.scalar.activation(gdump, xbf[:, hi - 2 * NW:hi],
                                         Act.Copy,
                                         accum_out=gsums[:, g // 2:g // 2 + 1])

            # ---- window prep (DVE): t1, mle, m2h, mu ----
            # t2 and m2v are applied as shifted-view matmuls on PE instead.
            T1s = [None] * NGRP
            M2Hs, MUPs = [None] * NGRP, [None] * NGRP

            def prep_window(g):
                G0 = g * NW
                t1 = win.tile([C, NW], bf16, tag="t1")
                mlp = wmid.tile([C, NW + 1], bf16, tag="mlp")
                mup = win.tile([C, NW + 128], bf16, tag="mup")
                m2h = win.tile([C, NW], bf16, tag="m2h")
                T1s[g] = t1
                M2Hs[g] = m2h
                MUPs[g] = mup

                # t1 = x(-1)+x(+1)
                ha = 1 if g == 0 else 0
                hb = NW - 1 if g == NGRP - 1 else NW
                nc.vector.tensor_tensor(t1[:, ha:hb],
                                        xbf[:, G0 + ha - 1:G0 + hb - 1],
                                        xbf[:, G0 + ha + 1:G0 + hb + 1],
                                        Alu.add)
                if g == 0:
                    nc.vector.tensor_copy(t1[:, 0:1], xbf[:, 1:2])
                if g == NGRP - 1:
                    nc.vector.tensor_copy(t1[:, NW - 1:NW],
                                          xbf[:, L - 2:L - 1])

                # mle window [G0, G0+NW] (NW+1 values)
                if g == 0:
                    nc.vector.memset(mlp[:, 0:1], 0.0)
                    nc.vector.tensor_tensor(mlp[:, 1:NW + 1],
                                            xbf[:, 1:NW + 1],
                                            xbf[:, 0:NW], Alu.min)
                elif g == NGRP - 1:
                    nc.vector.tensor_tensor(mlp[:, 0:NW],
                                            xbf[:, G0:L],
                                            xbf[:, G0 - 1:L - 1], Alu.min)
                    nc.vector.memset(mlp[:, NW:NW + 1], 0.0)
                else:
                    nc.vector.tensor_tensor(mlp[:, 0:NW + 1],
                                            xbf[:, G0:G0 + NW + 1],
                                            xbf[:, G0 - 1:G0 + NW], Alu.min)
                # m2h = mle + mle(+1)  (flat; edges fixed via Delta matmuls)
                nc.vector.tensor_tensor(m2h, mlp[:, 0:NW], mlp[:, 1:NW + 1],
                                        Alu.add)

                # mu window [G0, G0+NW+128)
                if g == 0:
                    nc.vector.memset(mup[:, 0:128], 0.0)
                    nc.vector.tensor_tensor(mup[:, 128:NW + 128],
                                            xbf[:, 128:NW + 128],
                                            xbf[:, 0:NW], Alu.min)
                elif g == NGRP - 1:
                    nc.vector.tensor_tensor(mup[:, 0:NW],
                                            xbf[:, G0:L],
                                            xbf[:, G0 - 128:L - 128], Alu.min)
                    nc.vector.memset(mup[:, NW:NW + 128], 0.0)
                else:
                    nc.vector.tensor_tensor(mup[:, 0:NW + 128],
                                            xbf[:, G0:G0 + NW + 128],
                                            xbf[:, G0 - 128:G0 + NW], Alu.min)

            # ---- h1 groups + main pair accumulation, interleaved ----
            def h1_group(k):
                h1ps = hps.tile([C, NCH], f32)
                for wgt, shift in [(mqt, 0), (mpt, -1), (mrt, +1)]:
                    for j in range(4):
                        n = 4 * k + j
                        n0 = n * NCH
                        lo = n0 + shift
                        hi = n0 + NCH + shift
                        plo, phi = 0, NCH
                        if lo < 0:
                            plo, lo = 1, 0
                        if hi > L:
                            phi, hi = NCH - 1, L
                        nc.tensor.matmul(
                            h1ps[32 * j:32 * j + 32, plo:phi],
                            wgt[:, 0:32], xbf[:, lo:hi],
                            start=(shift == 0), stop=(shift == 1),
                            tile_position=(0, 32 * j))
                nc.scalar.activation(h1sb[:, k * NCH:(k + 1) * NCH], h1ps,
                                     Act.Silu, bias=b1t[:, 0:1])

            _pair_ps = {}

            def pair_x(p):
                ps = yps.tile([C, 2 * NCH], f32)
                _pair_ps[p] = ps
                g = p // 2
                for half in range(2):
                    n = 2 * p + half
                    j = n % 4
                    n0 = n * NCH
                    off = (n - 4 * g) * NCH
                    c0 = half * NCH
                    nc.tensor.matmul(ps[:, c0:c0 + NCH], wb3t,
                                     xbf[:, n0:n0 + NCH],
                                     start=True, stop=False)
                    nc.tensor.matmul(ps[:, c0:c0 + NCH], wa2dt,
                                     T1s[g][:, off:off + NCH],
                                     start=False, stop=False)
                    # t2 as shifted x matmuls (zero-pad via clipped ranges)
                    a = 128 if n == 0 else 0
                    nc.tensor.matmul(ps[:, c0 + a:c0 + NCH], wa2dt,
                                     xbf[:, n0 + a - 128:n0 + NCH - 128],
                                     start=False, stop=False)
                    b = NCH - 128 if n == NCHUNK - 1 else NCH
                    nc.tensor.matmul(ps[:, c0:c0 + b], wa2dt,
                                     xbf[:, n0 + 128:n0 + 128 + b],
                                     start=False, stop=False)
                    nc.tensor.matmul(ps[:, c0:c0 + NCH], wdmt,
                                     M2Hs[g][:, off:off + NCH],
                                     start=False, stop=False)
                    # m2v as shifted mu matmuls
                    nc.tensor.matmul(ps[:, c0:c0 + NCH], wdmt,
                                     MUPs[g][:, off:off + NCH],
                                     start=False, stop=False)
                    nc.tensor.matmul(ps[:, c0:c0 + NCH], wdmt,
                                     MUPs[g][:, off + 128:off + 128 + NCH],
                                     start=False, stop=False)
                    if n == 0:
                        # col-scan wrap: l=c gets x[(h-1)w + c - 1]
                        nc.tensor.matmul(ps[:, 1:128], wa2t,
                                         xbf[:, L - Wimg:L - 1],
                                         start=False, stop=False)
                    if n == NCHUNK - 1:
                        # col-scan wrap: l=(h-1)w+c gets x[c+1]
                        nc.tensor.matmul(ps[:, 2 * NCH - 128:2 * NCH - 1],
                                         wa2t, xbf[:, 1:128],
                                         start=False, stop=False)

            def pair_fin(p):
                ps = _pair_ps.pop(p)
                g = p // 2
                ps3 = ps.rearrange("p (a b) -> p a b", b=Wimg)
                nc.tensor.matmul(ps3[:, :, 0:1], wdmt,
                                 d0t[:, 8 * p:8 * p + 8],
                                 start=False, stop=False)
                nc.tensor.matmul(ps3[:, :, 127:128], wdmt,
                                 d127t[:, 8 * p:8 * p + 8],
                                 start=False, stop=False)
                if p == 0:
                    # V-edge r=0: += Wdm @ (mu[l+128] - 0.5 x[l+128])
                    nc.tensor.matmul(ps[:, 0:128], wdmt, dv0t,
                                     start=False, stop=False)
                if p == NPAIR - 1:
                    # V-edge r=127: += Wdm @ (mu[l] - 0.5 x[l-128])
                    nc.tensor.matmul(ps[:, 2 * NCH - 128:2 * NCH], wdmt,
                                     dv1t, start=False, stop=False)
                for half in range(2):
                    n = 2 * p + half
                    j = n % 4
                    c0 = half * NCH
                    nc.tensor.matmul(ps[:, c0:c0 + NCH],
                                     c2t4[32 * j:32 * j + 32, :],
                                     h1sb[32 * j:32 * j + 32,
                                          g * NCH:(g + 1) * NCH],
                                     start=False, stop=True,
                                     tile_position=(32 * j, 0))
                # evacuate pair + channel sums
                p0 = p * 2 * NCH
                nc.scalar.activation(ypre[:, p0:p0 + 2 * NCH], ps,
                                     Act.Identity, bias=bout[:, 0:1],
                                     accum_out=ysum[:, p:p + 1])
                sqd = dump.tile([C, 2 * NCH], bf16, tag="sq")
                nc.scalar.activation(sqd, ypre[:, p0:p0 + 2 * NCH], Act.Square,
                                     accum_out=ysq[:, p:p + 1])

            def pair(p):
                pair_x(p)
                pair_fin(p)

            prep_window(0)
            prep_window(1)
            pair_x(2)
            pair_x(3)

            # ---- channel-branch small chain (needs all of x) ----
            gsum = sm.tile([C, 1], f32)
            nc.vector.tensor_reduce(gsum, gsums, mybir.AxisListType.X, Alu.add)
            ss_ps = sps.tile([1, 1], f32, tag="sp")
            nc.tensor.matmul(ss_ps, gsum, gsum, start=True, stop=True)
            ss = sm.tile([1, 1], f32)
            nc.vector.tensor_copy(ss, ss_ps)
            rn2 = sm.tile([1, 1], f32)
            nc.vector.reciprocal(rn2, ss)          # 1 / ||gsum||^2

            v_ps = sps.tile([C, 1], f32, tag="sp")
            nc.tensor.matmul(v_ps, wcho, gsum, start=True, stop=True)
            v_sb = sm.tile([C, 1], f32)
            nc.vector.tensor_copy(v_sb, v_ps)
            pqr = sm.tile([C, 3], f32)
            for j in range(3):
                nc.vector.tensor_tensor(pqr[:, j:j + 1], v_sb, taps[:, j:j + 1],
                                        Alu.mult)
            pqr2_ps = sps.tile([C, 3], f32, tag="sp")
            nc.tensor.matmul(pqr2_ps, wchi, pqr, start=True, stop=True)
            pqr2 = sm.tile([C, 3], f32)
            nc.vector.tensor_copy(pqr2, pqr2_ps)

            u_ps = sps.tile([1, 32], f32, tag="sp")
            nc.tensor.matmul(u_ps, gsum, wm1t, start=True, stop=True)
            u_sb = sm.tile([1, 32], f32)
            nc.vector.tensor_copy(u_sb, u_ps)
            u_sc = sm.tile([1, 32], f32)
            nc.vector.tensor_scalar(u_sc, u_sb, rn2[0:1, 0:1], None, Alu.mult)
            # broadcast u to all partitions: ones[C,1] (x) u  via PE
            ones1c = sm.tile([1, C], f32)
            nc.vector.memset(ones1c, 1.0)
            u_bc = sps.tile([C, 32], f32, tag="sp")
            nc.tensor.matmul(u_bc, ones1c, u_sc, start=True, stop=True)

            mqt = sm.tile([C, 32], bf16)
            mpt = sm.tile([C, 32], bf16)
            mrt = sm.tile([C, 32], bf16)
            for t, j in [(mpt, 0), (mqt, 1), (mrt, 2)]:
                nc.vector.tensor_scalar(t, u_bc, pqr2[:, j:j + 1], None,
                                        Alu.mult)

            # ---- H-edge Delta columns (c=0 / c=127 corrections) ----
            x3f = xbf.rearrange("p (r c) -> p r c", c=Wimg)
            mn01 = sm.tile([C, 128], bf16)
            mna = sm.tile([C, 129], bf16)
            mnf = sm.tile([C, 128], bf16)
            dd1 = sm.tile([C, 128], bf16)
            dd2 = sm.tile([C, 128], bf16)
            d0t = sm.tile([C, 128], bf16)
            d127t = sm.tile([C, 128], bf16)

            def r3(t):
                return t.rearrange("p (r c) -> p r c", c=1)

            nc.vector.tensor_tensor(r3(mn01), x3f[:, :, 0:1], x3f[:, :, 1:2],
                                    Alu.min)
            nc.vector.memset(mna[:, 0:1], 0.0)
            nc.vector.memset(mna[:, 128:129], 0.0)
            nc.vector.tensor_tensor(r3(mna[:, 1:128]), x3f[:, 1:128, 0:1],
                                    x3f[:, 0:127, 127:128], Alu.min)
            nc.vector.tensor_tensor(r3(mnf), x3f[:, :, 126:127],
                                    x3f[:, :, 127:128], Alu.min)
            nc.vector.tensor_copy(r3(dd1[:, 0:1]), x3f[:, 0:1, 1:2])
            nc.vector.tensor_tensor(r3(dd1[:, 1:128]), x3f[:, 1:128, 1:2],
                                    x3f[:, 0:127, 127:128], Alu.subtract)
            nc.vector.tensor_tensor(r3(dd2[:, 0:127]), x3f[:, 1:128, 0:1],
                                    x3f[:, 0:127, 126:127], Alu.subtract)
            nc.vector.tensor_scalar(r3(dd2[:, 127:128]),
                                    x3f[:, 127:128, 126:127], -1.0, None,
                                    Alu.mult)
            nc.vector.tensor_tensor(d0t, mn01, mna[:, 0:128], Alu.subtract)
            nc.vector.scalar_tensor_tensor(d0t, dd1, -0.5, d0t,
                                           Alu.mult, Alu.add)
            nc.vector.tensor_tensor(d127t, mnf, mna[:, 1:129], Alu.subtract)
            nc.vector.scalar_tensor_tensor(d127t, dd2, 0.5, d127t,
                                           Alu.mult, Alu.add)
            # V-edge Delta rows: dv0 = mu[l+128]-0.5x[l+128] (r=0),
            # dv1 = mu[l]-0.5x[l-128] (r=127); contiguous [C,128]
            dv0t = sm.tile([C, 128], bf16)
            dv1t = sm.tile([C, 128], bf16)
            nc.vector.tensor_tensor(dv0t, xbf[:, 128:256], xbf[:, 0:128],
                                    Alu.min)
            nc.vector.scalar_tensor_tensor(dv0t, xbf[:, 128:256], -0.5, dv0t,
                                           Alu.mult, Alu.add)
            nc.vector.tensor_tensor(dv1t, xbf[:, L - 128:L],
                                    xbf[:, L - 256:L - 128], Alu.min)
            nc.vector.scalar_tensor_tensor(dv1t, xbf[:, L - 256:L - 128],
                                           -0.5, dv1t, Alu.mult, Alu.add)

            for g in range(2, NGRP):
                prep_window(g)

            # pairs 2-4 x-parts run while the channel chain is pending
            pair_x(4)
            h1_group(0)
            h1_group(1)
            pair_fin(2)
            pair_fin(3)
            h1_group(2)
            pair_fin(4)
            pair(5)
            for k in range(3, NGRP):
                h1_group(k)
                pair(2 * k)
                pair(2 * k + 1)
            pair(0)
            pair(1)

            # ---- global BN stats via AllReduce ----
            # ysq tracked z = y - bout (PSUM, pre-bias):
            #   sum(y^2) = sum(z^2) + 2*bout*sum(y) - L*bout^2
            stats = sm.tile([C, 2], f32)
            nc.vector.tensor_reduce(stats[:, 0:1], ysum, mybir.AxisListType.X,
                                    Alu.add)
            nc.vector.tensor_reduce(stats[:, 1:2], ysq, mybir.AxisListType.X,
                                    Alu.add)
            # prefetch the sqrt ACT table while the collective runs
            sqpre = sm.tile([C, 1], f32)
            nc.scalar.activation(sqpre, stats[:, 1:2], Act.Sqrt)
            cc_in = dram.tile([C, 2], f32)
            cc_out = dram.tile([C, 2], f32)
            nc.gpsimd.dma_start(out=cc_in[:], in_=stats)
            nc.gpsimd.collective_compute(
                "AllReduce", Alu.add,
                replica_groups=[list(range(NCORES))],
                ins=[cc_in.opt()], outs=[cc_out.opt()])
            statsr = sm.tile([C, 2], f32)
            nc.gpsimd.dma_start(out=statsr, in_=cc_out[:])

            mean = sm.tile([C, 1], f32)
            ex2 = sm.tile([C, 1], f32)
            nc.vector.tensor_scalar(mean, statsr[:, 0:1], 1.0 / NTOT, None,
                                    Alu.mult)
            nc.vector.tensor_scalar(ex2, statsr[:, 1:2], 1.0 / NTOT, None,
                                    Alu.mult)
            m2 = sm.tile([C, 1], f32)
            nc.vector.tensor_tensor(m2, mean, mean, Alu.mult)
            varep = sm.tile([C, 1], f32)
            nc.vector.tensor_tensor(varep, ex2, m2, Alu.subtract)
            nc.vector.tensor_scalar(varep, varep, EPS_BN, None, Alu.add)
            inv = sm.tile([C, 1], f32)
            nc.vector.reciprocal(inv, varep)
            rstd = sm.tile([C, 1], f32)
            nc.scalar.activation(rstd, inv, Act.Sqrt)
            s_sc = sm.tile([C, 1], f32)
            nc.vector.tensor_tensor(s_sc, rstd, gb[:, 0:1], Alu.mult)
            ms = sm.tile([C, 1], f32)
            nc.vector.tensor_tensor(ms, mean, s_sc, Alu.mult)
            t_sc = sm.tile([C, 1], f32)
            nc.vector.tensor_tensor(t_sc, gb[:, 1:2], ms, Alu.subtract)

            # ---- apply BN into xbf (bf16), write out ----
            for g in range(NGRP):
                lo, hi = g * NW, (g + 1) * NW
                nc.vector.tensor_scalar(xbf[:, lo:hi], ypre[:, lo:hi],
                                        s_sc[:, 0:1], t_sc[:, 0:1],
                                        Alu.mult, Alu.add)
                if g % 2 == 1:
                    nc.sync.dma_start(out=y_ext[:, hi - 2 * NW:hi],
                                      in_=xbf[:, hi - 2 * NW:hi])

    _split_excess_waits(nc)
    return nc


def _fold_weights(inputs):
    f = np.float32
    W_in = inputs["w_spatial_in"].astype(np.float64)
    W_out = inputs["w_spatial_out"].astype(np.float64)
    dw_sp = inputs["w_dw_spatial"][:, 0, :].astype(np.float64)
    W_proj = inputs["w_out_proj"].astype(np.float64)
    W_mlp2 = inputs["w_mlp2"].astype(np.float64)
    dwt = float(inputs["diff_weight"])

    A2 = 0.25 * W_proj @ (W_out * (dw_sp[:, 0] + dw_sp[:, 2])[None, :]) @ W_in
    B3sp = W_proj @ (W_out * dw_sp[:, 1][None, :]) @ W_in
    Wd = 0.25 * dwt * W_proj
    B3pp = B3sp + W_proj + 4.0 * Wd
    A2d = A2 + Wd
    Wdm = -2.0 * Wd
    C2 = W_proj @ W_mlp2                     # [c, 32]
    bias_out = W_proj @ inputs["b_mlp2"].astype(np.float64)

    bf = ml_dtypes.bfloat16
    return {
        "wb3t": np.ascontiguousarray(B3pp.T.astype(bf)),
        "wa2dt": np.ascontiguousarray(A2d.T.astype(bf)),
        "wa2t": np.ascontiguousarray(A2.T.astype(bf)),
        "wdmt": np.ascontiguousarray(Wdm.T.astype(bf)),
        "c2t4": np.ascontiguousarray(np.tile(C2.T.astype(bf), (4, 1))),
        "wcho": np.ascontiguousarray(inputs["w_ch_out"].astype(f)),
        "wchi": np.ascontiguousarray(inputs["w_ch_in"].astype(f)),
        "wm1t": np.ascontiguousarray(inputs["w_mlp1"].T.astype(f)),
        "taps": np.ascontiguousarray(inputs["w_ch_dw"][:, 0, :].astype(f)),
        "b1t": np.ascontiguousarray(
            np.tile(inputs["b_mlp1"].astype(f), 4)[:, None]),
        "bout": np.ascontiguousarray(bias_out.astype(f)[:, None]),
        "gb": np.ascontiguousarray(
            np.stack([inputs["bn_gamma"], inputs["bn_beta"]], 1).astype(f)),
    }


def kernel(**inputs):
    from concourse.bass_utils import run_bass_kernel_spmd

    inputs = {k: np.asarray(v) for k, v in inputs.items()}
    if "nc" not in _CACHE:
        _CACHE["nc"] = _build_program()
    nc = _CACHE["nc"]

    wmap = _fold_weights(inputs)
    x = inputs["x"].astype(np.float32)  # [B, C, H, W]
    in_maps = []
    for b in range(NCORES):
        m = dict(wmap)
        m["x"] = np.ascontiguousarray(
            x[b].reshape(C, L).astype(ml_dtypes.bfloat16))
        in_maps.append(m)

    res = run_bass_kernel_spmd(nc, in_maps, list(range(NCORES)))
    out = np.stack([res.results[b]["y"].astype(np.float32)
                    .reshape(C, Himg, Wimg) for b in range(NCORES)])
    return out
